# revision 1
# baseline (speedup 1.0000x reference)
"""Bass/Trainium2 kernel for the DST-I spectral elliptic solver (QG model).

  psi = Cm2l @ [ S (Hinv * (S (Cl2m@q) S)) S  + alpha*homsol ]

S = symmetric orthogonal 2047-point DST-I matrix (padded to 2048). Each
1D transform uses the parity split  y[k] = v[k]+u[k], y[2046-k] = v[k]-u[k]
with v = x_even @ Q, u = x_odd @ P  (Q/P = even/odd rows of S, [1024x1024])
- half the matmul flops of a dense S multiply plus a cheap DVE butterfly.
Matmuls run in fp32r (the PE's full-rate reduced-precision fp32 mode).

Distribution: all 4 modes on every core; spatial axis sharded 8x256, chain
y -> kx -> ky -> x with one AllToAll per mode per hop (overlapped with
compute). Zero-mean correction via sum(psi) = w^T T w + tiny AllReduce.
"""
import numpy as np

NZ = 4          # layers/modes
N = 2047        # logical grid
NP = 2048       # padded grid
NH = 1024       # half grid
NC = 8          # cores
SH = NP // NC   # 256 shard width
HT = NH // 128  # 8 tiles per half axis
CH = NP // 512  # 4 chunks

_PROG = {}


def _build_program():
    import concourse.mybir as mybir
    import concourse.tile as tile
    from concourse import bacc

    F32 = mybir.dt.float32
    F32R = mybir.dt.float32r
    MUL = mybir.AluOpType.mult
    ADD = mybir.AluOpType.add
    SUB = mybir.AluOpType.subtract
    BYP = mybir.AluOpType.bypass
    RG = [list(range(NC))]

    nc = bacc.Bacc("TRN2", target_bir_lowering=False, debug=False, num_devices=NC)

    q_in = nc.dram_tensor("q_in", [NZ, NP, SH], F32, kind="ExternalInput")
    pq_in = nc.dram_tensor("pq_in", [2, HT, 128, NH], F32R, kind="ExternalInput")
    h_in = nc.dram_tensor("h_in", [NZ, 2, 128, NP], F32, kind="ExternalInput")
    hom_in = nc.dram_tensor("hom_in", [NZ, 2, 128, NP], F32, kind="ExternalInput")
    wrep_in = nc.dram_tensor("wrep_in", [128, NP], F32, kind="ExternalInput")
    wcol_in = nc.dram_tensor("wcol_in", [128, 2], F32, kind="ExternalInput")
    ones_in = nc.dram_tensor("ones_in", [1, 128], F32R, kind="ExternalInput")
    clm_in = nc.dram_tensor("clm_in", [128, 16], F32, kind="ExternalInput")
    cml_in = nc.dram_tensor("cml_in", [128, 16], F32, kind="ExternalInput")
    cmlT_in = nc.dram_tensor("cmlT_in", [4, 4], F32, kind="ExternalInput")
    hm_in = nc.dram_tensor("hm_in", [1, 4], F32, kind="ExternalInput")
    out_d = nc.dram_tensor("out_d", [NZ, 2, 128, NP], F32, kind="ExternalOutput")
    beta_d = nc.dram_tensor("beta_d", [4, 4], F32, kind="ExternalOutput")

    with tile.TileContext(nc) as tc:
        with (
            tc.tile_pool(name="dram", bufs=1, space="DRAM") as dram,
            tc.tile_pool(name="psum", bufs=8, space="PSUM") as psum,
            tc.tile_pool(name="const", bufs=1) as const,
            tc.tile_pool(name="tiny", bufs=1) as tiny,
        ):
            bin_ = [[dram.tile([NC, 2, 128, SH], F32R, tag=f"b{r}i{m}",
                               name=f"b{r}i{m}")
                     for m in range(NZ)] for r in range(3)]
            bout = [[dram.tile([NC, 2, 128, SH], F32R, tag=f"b{r}o{m}",
                               name=f"b{r}o{m}")
                     for m in range(NZ)] for r in range(3)]
            ar_i = dram.tile([1, 8], F32, tag="ar_i")
            ar_o = dram.tile([1, 8], F32, tag="ar_o")
            a4_d = dram.tile([1, 4], F32, tag="a4")
            b16_d = dram.tile([1, 16], F32R, tag="b16")
            e_d = dram.tile([NZ, 2, 128, NP], F32, tag="e_d")

            # Q (even rows) / P (odd rows) of S, resident in SBUF
            PQ = const.tile([128, 2, HT, NH], F32R, tag="PQ")
            for s in range(2):
                for k in range(HT):
                    nc.sync.dma_start(PQ[:, s, k, :], pq_in.ap()[s, k])
            wcol = const.tile([128, 2], F32, tag="wcol")
            nc.sync.dma_start(wcol[:], wcol_in.ap())
            ones = const.tile([1, 128], F32R, tag="ones")
            nc.sync.dma_start(ones[:], ones_in.ap())
            clm = const.tile([128, 16], F32, tag="clm")
            nc.sync.dma_start(clm[:], clm_in.ap())
            cml = const.tile([128, 16], F32, tag="cml")
            nc.sync.dma_start(cml[:], cml_in.ap())
            cmlT = const.tile([4, 4], F32, tag="cmlT")
            nc.sync.dma_start(cmlT[:], cmlT_in.ap())
            hm = const.tile([1, 4], F32, tag="hm")
            nc.sync.dma_start(hm[:], hm_in.ap())
            brep = const.tile([128, 16], F32, tag="brep")

            def a2a(src, dst):
                nc.gpsimd.collective_compute(
                    "AllToAll", BYP, replica_groups=RG,
                    ins=[src.opt()], outs=[dst.opt()])

            def load_parity(pool, tag, bsrc, name):
                """Gather even/odd rows of the 2048 axis from a bounce."""
                te = pool.tile([128, HT, SH], F32R, tag=tag, name=name + "e")
                to = pool.tile([128, HT, SH], F32R, tag=tag, name=name + "o")
                for pt in range(HT):
                    for par, t in ((0, te), (1, to)):
                        nc.sync.dma_start(
                            t[0:64, pt, :], bsrc[pt, 0, par:128:2, :])
                        nc.sync.dma_start(
                            t[64:128, pt, :], bsrc[pt, 1, par:128:2, :])
                return te, to

            with tc.tile_pool(name="kxp", bufs=4) as kxp:

                def split_stage(fe, fo, epilogue):
                    """One mode's transform: for each output subtile produce a
                    [128, 2048] natural-order staging, then epilogue(st, stg)."""
                    for st in range(2):
                        kxstg = kxp.tile([128, NP], F32R, tag="kx",
                                         name="kxstg")
                        for n2 in range(2):
                            pv = psum.tile([128, 512], F32, tag="acc",
                                           name=f"pv{n2}")
                            pu = psum.tile([128, 512], F32, tag="acc",
                                           name=f"pu{n2}")
                            for k in range(HT):
                                rq = PQ[:, 0, k, n2 * 512:(n2 + 1) * 512]
                                rp = PQ[:, 1, k, n2 * 512:(n2 + 1) * 512]
                                nc.tensor.matmul(
                                    pv[:], fe[:, k, st * 128:(st + 1) * 128],
                                    rq, start=(k == 0), stop=(k == HT - 1))
                                nc.tensor.matmul(
                                    pu[:], fo[:, k, st * 128:(st + 1) * 128],
                                    rp, start=(k == 0), stop=(k == HT - 1))
                            # butterfly: y[k]=v+u; y[2046-k]=v-u
                            # (DVE reads max one PSUM operand: stage u via SBUF)
                            usb = kxp.tile([128, 512], F32, tag="usb",
                                           name="usb")
                            nc.vector.tensor_copy(usb[:], pu[:])
                            nc.vector.tensor_tensor(
                                kxstg[:, n2 * 512:(n2 + 1) * 512],
                                pv[:], usb[:], ADD)
                            if n2 == 0:
                                nc.vector.tensor_tensor(
                                    kxstg[:, 1535:2047][:, ::-1],
                                    pv[:], usb[:], SUB)
                            else:
                                nc.vector.tensor_tensor(
                                    kxstg[:, 1024:1535][:, ::-1],
                                    pv[:, 0:511], usb[:, 0:511], SUB)
                        nc.vector.memset(kxstg[:, 2047:2048].bitcast(F32), 0.0)
                        epilogue(st, kxstg)

                def evict_to_bounce(bdst):
                    def ep(st, kxstg):
                        for d in range(NC):
                            nc.sync.dma_start(
                                bdst[d, st, :, :],
                                kxstg[:, d * 256:(d + 1) * 256])
                    return ep

                # ---------------- premix + stage 1 ----------------
                with (
                    tc.tile_pool(name="fpool", bufs=8) as fpool,
                    tc.tile_pool(name="qpool", bufs=10) as qpool,
                ):
                    f_e, f_o = {}, {}
                    for m in range(NZ):
                        f_e[m] = fpool.tile([128, HT, SH], F32R, tag="f",
                                            name=f"fe{m}")
                        f_o[m] = fpool.tile([128, HT, SH], F32R, tag="f",
                                            name=f"fo{m}")
                    for par in range(2):
                        for pt in range(HT):
                            qts = []
                            for l in range(NZ):
                                qt = qpool.tile([128, SH], F32, tag="q",
                                                name=f"q{par}{pt}{l}")
                                lo = 2 * pt * 128 + par
                                nc.sync.dma_start(
                                    qt[:],
                                    q_in.ap()[l, lo:min(lo + 256, NP):2, :])
                                qts.append(qt)
                            for m in range(NZ):
                                dst = (f_e, f_o)[par][m][:, pt, :]
                                nc.vector.tensor_scalar(
                                    dst, qts[0][:], clm[:, 4 * m:4 * m + 1],
                                    None, MUL)
                                for l in (1, 2, 3):
                                    nc.vector.scalar_tensor_tensor(
                                        dst, qts[l][:],
                                        clm[:, 4 * m + l:4 * m + l + 1],
                                        dst, MUL, ADD)
                    for m in range(NZ):
                        split_stage(f_e[m], f_o[m],
                                    evict_to_bounce(bin_[0][m]))
                        a2a(bin_[0][m], bout[0][m])

                # ---------------- stage 2 + Hinv + z ----------------
                with tc.tile_pool(name="stin", bufs=4) as stin:
                    psz = psum.tile([1, 4], F32, tag="acc")
                    with (
                        tc.tile_pool(name="hpool", bufs=3) as hpool,
                        tc.tile_pool(name="wpool", bufs=1) as wpool,
                        tc.tile_pool(name="zpool", bufs=12) as zpool,
                    ):
                        wrep = wpool.tile([128, NP], F32, tag="wrep")
                        nc.sync.dma_start(wrep[:], wrep_in.ap())
                        for m in range(NZ):
                            te, to = load_parity(stin, "stin", bout[0][m],
                                                 f"a{m}")
                            zprev = [None, None]

                            def ep2(st, kxstg, m=m, zprev=zprev):
                                for n in range(CH):
                                    sl = slice(n * 512, (n + 1) * 512)
                                    ht = hpool.tile([128, 512], F32, tag="ht",
                                                    name="ht")
                                    nc.sync.dma_start(
                                        ht[:], h_in.ap()[m, st, :, sl])
                                    hs_ = hpool.tile([128, 512], F32,
                                                     tag="hs", name="hs")
                                    hv = hpool.tile([128, 512], F32,
                                                    tag="hv", name="hv")
                                    nc.vector.reciprocal_approx_accurate(
                                        out=hv[:], in_=ht[:], scratch=hs_[:])
                                    nc.vector.tensor_tensor(
                                        kxstg[:, sl],
                                        kxstg[:, sl].bitcast(F32), hv[:], MUL)
                                    scr = hpool.tile([128, 512], F32,
                                                     tag="scr", name="scr")
                                    nc.vector.tensor_tensor(
                                        scr[:], kxstg[:, sl].bitcast(F32),
                                        wrep[:, sl], MUL)
                                    za = zpool.tile([128, 1], F32, tag="za",
                                                    name="za")
                                    nc.vector.reduce_sum(
                                        za[:], scr[:],
                                        axis=mybir.AxisListType.X)
                                    if zprev[st] is None:
                                        zprev[st] = za
                                    else:
                                        zn = zpool.tile([128, 1], F32,
                                                        tag="za", name="zn")
                                        nc.vector.tensor_tensor(
                                            zn[:], zprev[st][:], za[:], ADD)
                                        zprev[st] = zn
                                for d in range(NC):
                                    nc.sync.dma_start(
                                        bin_[1][m][d, st, :, :],
                                        kxstg[:, d * 256:(d + 1) * 256])

                            split_stage(te, to, ep2)
                            for st in range(2):
                                nc.tensor.matmul(
                                    psz[0:1, m:m + 1], wcol[:, st:st + 1],
                                    zprev[st][:],
                                    start=(st == 0), stop=(st == 1))
                            a2a(bin_[1][m], bout[1][m])

                        # ---- alpha / beta ----
                        z_sb = zpool.tile([1, 4], F32, tag="zsb")
                        nc.vector.tensor_copy(z_sb[:], psz[:])
                        nc.sync.dma_start(ar_i[0:1, 0:4], z_sb[:])
                        nc.sync.dma_start(ar_i[0:1, 4:8], z_sb[:])
                        nc.gpsimd.collective_compute(
                            "AllReduce", ADD, replica_groups=RG,
                            ins=[ar_i.opt()], outs=[ar_o.opt()])
                        zsum = tiny.tile([1, 4], F32, tag="zsum")
                        nc.sync.dma_start(zsum[:], ar_o[0:1, 0:4])
                        rh = tiny.tile([1, 4], F32, tag="rh")
                        nc.vector.reciprocal(rh[:], hm[:])
                        al = tiny.tile([1, 4], F32, tag="al")
                        nc.vector.tensor_tensor(al[:], zsum[:], rh[:], MUL)
                        nc.vector.tensor_scalar(
                            al[:], al[:], -1.0 / (NP * NP), None, MUL)
                        nc.sync.dma_start(a4_d[:], al[:])
                        alc = tiny.tile([4, 1], F32, tag="alc")
                        nc.sync.dma_start(alc[:],
                                          a4_d[:].rearrange("a b -> b a"))
                        bT = tiny.tile([4, 4], F32, tag="bT")
                        nc.vector.tensor_scalar(bT[:], cmlT[:], alc[:, 0:1],
                                                None, MUL)
                        nc.sync.dma_start(beta_d.ap(), bT[:])
                        for r in range(4):
                            nc.sync.dma_start(
                                b16_d[0:1, 4 * r:4 * r + 4],
                                bT[r:r + 1, :].bitcast(F32R))
                        brow = tiny.tile([1, 16], F32R, tag="brow")
                        nc.sync.dma_start(brow[:], b16_d[:])
                        bps = psum.tile([128, 16], F32, tag="acc")
                        nc.tensor.matmul(bps[:], ones[:], brow[:],
                                         start=True, stop=True)
                        nc.vector.tensor_copy(brep[:], bps[:])

                    # ------------- stage 3 + E precompute -------------
                    with tc.tile_pool(name="epool", bufs=4) as epool:
                        for m in range(NZ):
                            te, to = load_parity(stin, "stin", bout[1][m],
                                                 f"t{m}")
                            split_stage(te, to, evict_to_bounce(bin_[2][m]))
                            a2a(bin_[2][m], bout[2][m])
                            xt = m // 2
                            for n in (2 * (m % 2), 2 * (m % 2) + 1):
                                sl = slice(n * 512, (n + 1) * 512)
                                hts = []
                                for mm in range(NZ):
                                    h_t = epool.tile([128, 512], F32,
                                                     tag="eh", name=f"eh{mm}")
                                    nc.sync.dma_start(
                                        h_t[:], hom_in.ap()[mm, xt, :, sl])
                                    hts.append(h_t)
                                for l in range(NZ):
                                    et = epool.tile([128, 512], F32,
                                                    tag="et", name=f"et{l}")
                                    nc.vector.tensor_scalar(
                                        et[:], hts[0][:], brep[:, l:l + 1],
                                        None, MUL)
                                    for mm in (1, 2, 3):
                                        nc.vector.scalar_tensor_tensor(
                                            et[:], hts[mm][:],
                                            brep[:, 4 * mm + l:
                                                 4 * mm + l + 1],
                                            et[:], MUL, ADD)
                                    nc.sync.dma_start(e_d[l, xt, :, sl],
                                                      et[:])
                                    if n == 3:
                                        nc.sync.dma_start(
                                            out_d.ap()[l, xt, :, 2047:2048],
                                            et[:, 511:512])

            # ---------------- stage 4 + postmix ----------------
            with (
                tc.tile_pool(name="u4", bufs=8) as u4,
                tc.tile_pool(name="pstg", bufs=10) as pstg,
                tc.tile_pool(name="tmppool", bufs=10) as tmp,
                tc.tile_pool(name="erp", bufs=4) as erp,
            ):
                ures = []
                for m in range(NZ):
                    ures.append(load_parity(u4, "u4", bout[2][m], f"u{m}"))
                for st in range(2):
                    for n2 in range(2):
                        pv, pu = [], []
                        for m in range(NZ):
                            pv.append(psum.tile([128, 512], F32, tag="acc",
                                                name=f"s4v{m}"))
                            pu.append(psum.tile([128, 512], F32, tag="acc",
                                                name=f"s4u{m}"))
                        for m in range(NZ):
                            te, to = ures[m]
                            for k in range(HT):
                                rq = PQ[:, 0, k, n2 * 512:(n2 + 1) * 512]
                                rp = PQ[:, 1, k, n2 * 512:(n2 + 1) * 512]
                                nc.tensor.matmul(
                                    pv[m][:],
                                    te[:, k, st * 128:(st + 1) * 128], rq,
                                    start=(k == 0), stop=(k == HT - 1))
                                nc.tensor.matmul(
                                    pu[m][:],
                                    to[:, k, st * 128:(st + 1) * 128], rp,
                                    start=(k == 0), stop=(k == HT - 1))
                        pss, psd = [], []
                        for m in range(NZ):
                            usb = pstg.tile([128, 512], F32, tag="pst",
                                            name=f"usb{m}")
                            nc.vector.tensor_copy(usb[:], pu[m][:])
                            a = pstg.tile([128, 512], F32, tag="pst",
                                          name=f"pss{m}")
                            nc.vector.tensor_tensor(a[:], pv[m][:], usb[:],
                                                    ADD)
                            pss.append(a)
                            b = pstg.tile([128, 512], F32, tag="pst",
                                          name=f"psd{m}")
                            nc.vector.tensor_tensor(b[:], pv[m][:], usb[:],
                                                    SUB)
                            psd.append(b)
                        # sum side -> cols [n2*512, n2*512+512)
                        sl = slice(n2 * 512, (n2 + 1) * 512)
                        for l in range(NZ):
                            t = tmp.tile([128, 512], F32, tag="tmp",
                                         name=f"ts{l}")
                            nc.vector.tensor_scalar(
                                t[:], pss[0][:], cml[:, l:l + 1], None, MUL)
                            for m in (1, 2, 3):
                                nc.vector.scalar_tensor_tensor(
                                    t[:], pss[m][:],
                                    cml[:, 4 * m + l:4 * m + l + 1],
                                    t[:], MUL, ADD)
                            et = erp.tile([128, 512], F32, tag="er",
                                          name=f"ers{l}")
                            nc.sync.dma_start(et[:], e_d[l, st, :, sl])
                            nc.vector.tensor_tensor(t[:], t[:], et[:], ADD)
                            nc.sync.dma_start(out_d.ap()[l, st, :, sl], t[:])
                        # diff side: n2=0 -> pos 2046..1535 (k=0..511)
                        #            n2=1 -> pos 1534..1024 (k=512..1022)
                        cnt = 512 if n2 == 0 else 511
                        dsl = (slice(1535, 2047) if n2 == 0
                               else slice(1024, 1535))
                        for l in range(NZ):
                            t = tmp.tile([128, cnt], F32, tag="tmp",
                                         name=f"td{l}")
                            nc.vector.tensor_scalar(
                                t[:], psd[0][:, 0:cnt], cml[:, l:l + 1],
                                None, MUL)
                            for m in (1, 2, 3):
                                nc.vector.scalar_tensor_tensor(
                                    t[:], psd[m][:, 0:cnt],
                                    cml[:, 4 * m + l:4 * m + l + 1],
                                    t[:], MUL, ADD)
                            et = erp.tile([128, cnt], F32, tag="er",
                                          name=f"erd{l}")
                            nc.sync.dma_start(et[:], e_d[l, st, :, dsl])
                            tf = tmp.tile([128, cnt], F32, tag="tmp",
                                          name=f"tf{l}")
                            nc.vector.tensor_tensor(
                                tf[:], t[:, ::-1], et[:], ADD)
                            nc.sync.dma_start(out_d.ap()[l, st, :, dsl],
                                              tf[:])
    nc.compile()
    return nc


def _host_prep(q, Cl2m, Cm2l, H, homsol, homsol_mean):
    f32 = np.float32
    k = np.arange(1, NP, dtype=np.float64)
    S = np.sqrt(2.0 / NP) * np.sin(np.pi / NP * np.outer(k, k))
    Spad = np.zeros((NP, NP), f32)
    Spad[:N, :N] = S.astype(f32)
    w = np.zeros(NP, f32)
    w[:N] = S.sum(axis=0).astype(f32)

    # pq_in[0] = Q (even rows), pq_in[1] = P (odd rows)
    pq = np.stack([Spad[0::2, :NH], Spad[1::2, :NH]])
    pq = np.ascontiguousarray(pq.reshape(2, HT, 128, NH))

    qp = np.zeros((NZ, NP, NP), f32)
    qp[:, :N, :N] = q
    Hp = np.ones((NZ, NP, NP), f32)
    Hp[:, :N, :N] = H
    hom = np.ascontiguousarray(homsol[:, 1:NP + 1, 1:NP + 1])

    wrep = np.broadcast_to(w, (128, NP)).copy()
    ones_r = np.ones((1, 128), f32)
    clm = np.broadcast_to(Cl2m.reshape(1, 16), (128, 16)).copy().astype(f32)
    cml = np.broadcast_to(Cm2l.T.reshape(1, 16), (128, 16)).copy().astype(f32)
    cmlT = np.ascontiguousarray(Cm2l.T).astype(f32)
    hm_i = homsol_mean.reshape(1, 4).astype(f32)

    in_maps = []
    for c in range(NC):
        ys = slice(c * SH, (c + 1) * SH)
        in_maps.append({
            "q_in": np.ascontiguousarray(qp[:, :, ys]),
            "pq_in": pq,
            "h_in": np.ascontiguousarray(Hp[:, ys, :]).reshape(NZ, 2, 128, NP),
            "hom_in": np.ascontiguousarray(
                hom[:, ys, :]).reshape(NZ, 2, 128, NP),
            "wrep_in": wrep,
            "wcol_in": np.ascontiguousarray(w[ys].reshape(2, 128).T),
            "ones_in": ones_r,
            "clm_in": clm,
            "cml_in": cml,
            "cmlT_in": cmlT,
            "hm_in": hm_i,
        })
    return in_maps


def kernel(q, Cl2m, Cm2l, helmholtz_mat, homsol, homsol_mean,
           _want_results=False):
    from concourse.bass_utils import run_bass_kernel_spmd

    if "nc" not in _PROG:
        _PROG["nc"] = _build_program()
    nc = _PROG["nc"]

    in_maps = _host_prep(np.asarray(q, np.float32),
                         np.asarray(Cl2m, np.float32),
                         np.asarray(Cm2l, np.float32),
                         np.asarray(helmholtz_mat, np.float32),
                         np.asarray(homsol, np.float32),
                         np.asarray(homsol_mean, np.float32))
    res = run_bass_kernel_spmd(nc, in_maps, core_ids=list(range(NC)),
                               **_PROG.get("run_kwargs", {}))
    out = np.zeros((NZ, NP + 1, NP + 1), np.float32)
    for c in range(NC):
        core = res.results[c]["out_d"].reshape(NZ, SH, NP)
        out[:, 1 + c * SH:1 + (c + 1) * SH, 1:] = core
    beta = res.results[0]["beta_d"]  # beta[m, l] = alpha_m * Cm2l[l, m]
    hs = np.asarray(homsol, np.float32)
    out[:, 0, :] = np.einsum("ml,my->ly", beta, hs[:, 0, :])
    out[:, 1:, 0] = np.einsum("ml,mx->lx", beta, hs[:, 1:, 0])
    if _want_results:
        return out, res
    return out



# revision 13
# speedup vs baseline: 2.0169x; 2.0169x over previous
"""Bass/Trainium2 kernel for the DST-I spectral elliptic solver (QG model).

  psi = Cm2l @ [ S (Hinv * (S (Cl2m@q) S)) S  + alpha*homsol ]

S = symmetric orthogonal 2047-point DST-I matrix (padded to 2048). Each
1D transform uses the parity split  y[k] = v[k]+u[k], y[2046-k] = v[k]-u[k]
with v = x_even @ Q, u = x_odd @ P  (Q/P = even/odd rows of S, [1024x1024]).
Transform data runs in bf16 (full-rate PE, half DMA traffic); accumulation
stays fp32 in PSUM.

Distribution: all 4 modes on every core; spatial axis sharded 8x256, chain
y -> kx -> ky -> x with one AllToAll per mode per hop (overlapped with
compute). Zero-mean correction via sum(psi) = w^T T w + tiny AllReduce,
fired right after stage 2 and consumed after stage 3 (no pipeline bubble).
Elementwise work is split across Vector/GpSimd/Scalar engines.
"""
import numpy as np

NZ = 4          # layers/modes
N = 2047        # logical grid
NP = 2048       # padded grid
NH = 1024       # half grid
NC = 8          # cores
SH = NP // NC   # 256 shard width
HT = NH // 128  # 8 tiles per half axis

_PROG = {}


def _build_program():
    import concourse.mybir as mybir
    import concourse.tile as tile
    from concourse import bacc

    F32 = mybir.dt.float32
    F32R = mybir.dt.float32r
    BF16 = mybir.dt.bfloat16
    MUL = mybir.AluOpType.mult
    ADD = mybir.AluOpType.add
    SUB = mybir.AluOpType.subtract
    BYP = mybir.AluOpType.bypass
    RG = [list(range(NC))]

    nc = bacc.Bacc("TRN2", target_bir_lowering=False, debug=False, num_devices=NC)

    q_in = nc.dram_tensor("q_in", [NZ, 2, HT, 128, SH], F32, kind="ExternalInput")
    pq_in = nc.dram_tensor("pq_in", [2, HT, 128, NH], BF16, kind="ExternalInput")
    h_in = nc.dram_tensor("h_in", [NZ, 2, 128, NP], BF16, kind="ExternalInput")
    hom_in = nc.dram_tensor("hom_in", [NZ, 2, 128, NP], BF16,
                            kind="ExternalInput")
    wrep_in = nc.dram_tensor("wrep_in", [128, NP], BF16, kind="ExternalInput")
    wcol_in = nc.dram_tensor("wcol_in", [128, 2], F32, kind="ExternalInput")
    ones_in = nc.dram_tensor("ones_in", [1, 128], F32R, kind="ExternalInput")
    clm_in = nc.dram_tensor("clm_in", [128, 16], F32, kind="ExternalInput")
    cml_in = nc.dram_tensor("cml_in", [128, 16], F32, kind="ExternalInput")
    cmlT_in = nc.dram_tensor("cmlT_in", [4, 4], F32, kind="ExternalInput")
    hm_in = nc.dram_tensor("hm_in", [1, 4], F32, kind="ExternalInput")
    out_d = nc.dram_tensor("out_d", [NZ, 2, 128, NP], F32, kind="ExternalOutput")
    beta_d = nc.dram_tensor("beta_d", [4, 4], F32, kind="ExternalOutput")

    with tile.TileContext(nc) as tc:
        with (
            tc.tile_pool(name="dram", bufs=1, space="DRAM") as dram,
            tc.tile_pool(name="psum", bufs=8, space="PSUM") as psum,
            tc.tile_pool(name="const", bufs=1) as const,
            tc.tile_pool(name="tiny", bufs=1) as tiny,
        ):
            bin_ = [[dram.tile([NC, 2, 128, SH], BF16, tag=f"b{r}i{m}",
                               name=f"b{r}i{m}")
                     for m in range(NZ)] for r in range(3)]
            bout = [[dram.tile([NC, 2, 128, SH], BF16, tag=f"b{r}o{m}",
                               name=f"b{r}o{m}")
                     for m in range(NZ)] for r in range(3)]
            ar_i = dram.tile([1, 8], F32, tag="ar_i")
            ar_o = dram.tile([1, 8], F32, tag="ar_o")
            a4_d = dram.tile([1, 4], F32, tag="a4")
            b16_d = dram.tile([1, 16], F32R, tag="b16")

            # Q (even rows) / P (odd rows) of S, resident in SBUF (bf16)
            PQ = const.tile([128, 2, HT, NH], BF16, tag="PQ")
            for s in range(2):
                for k in range(HT):
                    nc.sync.dma_start(PQ[:, s, k, :], pq_in.ap()[s, k])
            wcol = const.tile([128, 2], F32, tag="wcol")
            nc.sync.dma_start(wcol[:], wcol_in.ap())
            ones = const.tile([1, 128], F32R, tag="ones")
            nc.sync.dma_start(ones[:], ones_in.ap())
            clm = const.tile([128, 16], F32, tag="clm")
            nc.sync.dma_start(clm[:], clm_in.ap())
            cml = const.tile([128, 16], F32, tag="cml")
            nc.sync.dma_start(cml[:], cml_in.ap())
            cmlT = const.tile([4, 4], F32, tag="cmlT")
            nc.sync.dma_start(cmlT[:], cmlT_in.ap())
            hm = const.tile([1, 4], F32, tag="hm")
            nc.sync.dma_start(hm[:], hm_in.ap())
            wrep = const.tile([128, NP], BF16, tag="wrep")
            nc.sync.dma_start(wrep[:], wrep_in.ap())
            brep = const.tile([128, 16], F32, tag="brep")
            zcol = const.tile([128, 1], F32, tag="zcol")
            nc.vector.memset(zcol[:], 0.0)

            def a2a(src, dst):
                nc.gpsimd.collective_compute(
                    "AllToAll", BYP, replica_groups=RG,
                    ins=[src.opt()], outs=[dst.opt()])

            def load_parity(pool, tag, bsrc, name):
                """Gather even/odd rows of the 2048 axis from a bounce."""
                te = pool.tile([128, HT, SH], BF16, tag=tag, name=name + "e")
                to = pool.tile([128, HT, SH], BF16, tag=tag, name=name + "o")
                for par, t in ((0, te), (1, to)):
                    nc.sync.dma_start(
                        t[0:64, :, :],
                        bsrc[0:NC, 0, par:128:2, :].rearrange("d p c -> p d c"))
                    nc.sync.dma_start(
                        t[64:128, :, :],
                        bsrc[0:NC, 1, par:128:2, :].rearrange("d p c -> p d c"))
                return te, to

            with (
                tc.tile_pool(name="kxp", bufs=4) as kxp,
                tc.tile_pool(name="bfp", bufs=8) as bfp,
            ):

                def split_stage(fe, fo, epilogue):
                    """One mode's transform: for each output subtile produce a
                    [128, 2048] natural-order staging, then epilogue(st, stg)."""
                    for st in range(2):
                        kxstg = kxp.tile([128, NP], BF16, tag="kx",
                                         name="kxstg")
                        for n2 in range(2):
                            pv = psum.tile([128, 512], F32, tag="acc",
                                           name=f"pv{n2}")
                            pu = psum.tile([128, 512], F32, tag="acc",
                                           name=f"pu{n2}")
                            for k in range(HT):
                                rq = PQ[:, 0, k, n2 * 512:(n2 + 1) * 512]
                                rp = PQ[:, 1, k, n2 * 512:(n2 + 1) * 512]
                                nc.tensor.matmul(
                                    pv[:], fe[:, k, st * 128:(st + 1) * 128],
                                    rq, start=(k == 0), stop=(k == HT - 1))
                                nc.tensor.matmul(
                                    pu[:], fo[:, k, st * 128:(st + 1) * 128],
                                    rp, start=(k == 0), stop=(k == HT - 1))
                            # butterfly: y[k]=v+u; y[2046-k]=v-u
                            # scalar engine evacuates PSUM -> bf16, then the
                            # packed-bf16 add/sub run on vector/gpsimd
                            vsb = bfp.tile([128, 512], BF16, tag="bfly",
                                           name="vsb")
                            usb = bfp.tile([128, 512], BF16, tag="bfly",
                                           name="usb")
                            nc.scalar.copy(vsb[:], pv[:])
                            nc.scalar.copy(usb[:], pu[:])
                            nc.vector.tensor_tensor(
                                kxstg[:, n2 * 512:(n2 + 1) * 512],
                                vsb[:], usb[:], ADD)
                            if n2 == 0:
                                nc.vector.tensor_tensor(
                                    kxstg[:, 1535:2047][:, ::-1],
                                    vsb[:], usb[:], SUB)
                            else:
                                nc.vector.tensor_tensor(
                                    kxstg[:, 1024:1535][:, ::-1],
                                    vsb[:, 0:511], usb[:, 0:511], SUB)
                        nc.vector.tensor_copy(kxstg[:, 2047:2048], zcol[:])
                        epilogue(st, kxstg)

                def evict_to_bounce(bdst):
                    def ep(st, kxstg):
                        nc.sync.dma_start(
                            bdst[0:NC, st, :, :].rearrange("d p c -> p d c"),
                            kxstg[:].rearrange("p (d c) -> p d c", d=NC))
                    return ep

                # ---------------- premix + stage 1 ----------------
                with (
                    tc.tile_pool(name="fpool", bufs=8) as fpool,
                    tc.tile_pool(name="qpool", bufs=8) as qpool,
                    tc.tile_pool(name="mxs", bufs=4) as mxs,
                ):
                    f_e, f_o = {}, {}
                    for m in range(NZ):
                        f_e[m] = fpool.tile([128, HT, SH], BF16, tag="f",
                                            name=f"fe{m}")
                        f_o[m] = fpool.tile([128, HT, SH], BF16, tag="f",
                                            name=f"fo{m}")
                    qt = {}
                    for par in range(2):
                        for l in range(NZ):
                            t = qpool.tile([128, HT, SH], F32, tag="q",
                                           name=f"q{par}{l}")
                            nc.sync.dma_start(
                                t[:, :, :],
                                q_in.ap()[l, par].rearrange("k p c -> p k c"))
                            qt[(par, l)] = t
                    def premix(m):
                        # layer -> mode combine; head on Scalar, madds on DVE
                        for par in range(2):
                            dst = (f_e, f_o)[par][m]
                            acc = mxs.tile([128, HT, SH], F32, tag="mx",
                                           name=f"mx{par}")
                            nc.scalar.mul(acc[:, :, :], qt[(par, 0)][:, :, :],
                                          clm[:, 4 * m:4 * m + 1])
                            for l in (1, 2):
                                nc.vector.scalar_tensor_tensor(
                                    acc[:, :, :], qt[(par, l)][:, :, :],
                                    clm[:, 4 * m + l:4 * m + l + 1],
                                    acc[:, :, :], MUL, ADD)
                            nc.vector.scalar_tensor_tensor(
                                dst[:, :, :], qt[(par, 3)][:, :, :],
                                clm[:, 4 * m + 3:4 * m + 4],
                                acc[:, :, :], MUL, ADD)

                    # interleave premix with stage 1 so the in-order DVE
                    # queue always has ready work
                    premix(0)
                    premix(1)
                    for m in range(NZ):
                        if m + 2 < NZ:
                            premix(m + 2)
                        split_stage(f_e[m], f_o[m],
                                    evict_to_bounce(bin_[0][m]))
                        a2a(bin_[0][m], bout[0][m])

                # ---------------- stage 2 + Hinv + z ----------------
                with tc.tile_pool(name="stin", bufs=4) as stin:
                    psz = psum.tile([1, 4], F32, tag="acc")
                    with (
                        tc.tile_pool(name="hpool", bufs=3) as hpool,
                        tc.tile_pool(name="zpool", bufs=8) as zpool,
                    ):
                        for m in range(NZ):
                            te, to = load_parity(stin, "stin", bout[0][m],
                                                 f"a{m}")
                            zs = []

                            def ep2(st, kxstg, m=m, zs=zs):
                                ht = hpool.tile([128, NP], BF16, tag="ht",
                                                name="ht")
                                nc.sync.dma_start(ht[:], h_in.ap()[m, st])
                                nc.vector.tensor_tensor(
                                    kxstg[:], kxstg[:], ht[:], MUL)
                                junk = hpool.tile([128, NP], F32, tag="junk",
                                                  name="junk")
                                nc.vector.tensor_tensor(
                                    junk[:], kxstg[:], wrep[:], MUL)
                                za = zpool.tile([128, 1], F32, tag="za",
                                                name=f"za{st}")
                                nc.vector.reduce_sum(
                                    za[:], junk[:],
                                    axis=mybir.AxisListType.X)
                                zs.append(za)
                                nc.sync.dma_start(
                                    bin_[1][m][0:NC, st, :, :].rearrange(
                                        "d p c -> p d c"),
                                    kxstg[:].rearrange("p (d c) -> p d c",
                                                       d=NC))

                            split_stage(te, to, ep2)
                            for st in range(2):
                                nc.tensor.matmul(
                                    psz[0:1, m:m + 1], wcol[:, st:st + 1],
                                    zs[st][:],
                                    start=(st == 0), stop=(st == 1))
                            a2a(bin_[1][m], bout[1][m])

                        # ---- fire the AllReduce now; consume after s3 ----
                        z_sb = zpool.tile([1, 4], F32, tag="zsb")
                        nc.vector.tensor_copy(z_sb[:], psz[:])
                        nc.scalar.dma_start(ar_i[0:1, 0:4], z_sb[:])
                        nc.scalar.dma_start(ar_i[0:1, 4:8], z_sb[:])
                        nc.gpsimd.collective_compute(
                            "AllReduce", ADD, replica_groups=RG,
                            ins=[ar_i.opt()], outs=[ar_o.opt()])

                    # ------------- stage 3 (transforms only) -------------
                    for m in range(NZ):
                        te, to = load_parity(stin, "stin", bout[1][m],
                                             f"t{m}")
                        split_stage(te, to, evict_to_bounce(bin_[2][m]))
                        a2a(bin_[2][m], bout[2][m])

                # ---- alpha / beta + homogeneous correction E (SBUF) ----
                with (
                    tc.tile_pool(name="epool", bufs=1) as epool,
                    tc.tile_pool(name="homp", bufs=8) as homp,
                ):
                    zsum = tiny.tile([1, 4], F32, tag="zsum")
                    nc.scalar.dma_start(zsum[:], ar_o[0:1, 0:4])
                    rh = tiny.tile([1, 4], F32, tag="rh")
                    nc.vector.reciprocal(rh[:], hm[:])
                    al = tiny.tile([1, 4], F32, tag="al")
                    nc.vector.tensor_tensor(al[:], zsum[:], rh[:], MUL)
                    nc.vector.tensor_scalar(
                        al[:], al[:], -1.0 / (NP * NP), None, MUL)
                    nc.scalar.dma_start(a4_d[:], al[:])
                    alc = tiny.tile([4, 1], F32, tag="alc")
                    nc.scalar.dma_start(alc[:],
                                        a4_d[:].rearrange("a b -> b a"))
                    bT = tiny.tile([4, 4], F32, tag="bT")
                    nc.vector.tensor_scalar(bT[:], cmlT[:], alc[:, 0:1],
                                            None, MUL)
                    nc.scalar.dma_start(beta_d.ap(), bT[:])
                    for r in range(4):
                        nc.scalar.dma_start(
                            b16_d[0:1, 4 * r:4 * r + 4],
                            bT[r:r + 1, :].bitcast(F32R))
                    brow = tiny.tile([1, 16], F32R, tag="brow")
                    nc.scalar.dma_start(brow[:], b16_d[:])
                    bps = psum.tile([128, 16], F32, tag="acc")
                    nc.tensor.matmul(bps[:], ones[:], brow[:],
                                     start=True, stop=True)
                    nc.vector.tensor_copy(brep[:], bps[:])

                    # E[l, st] = sum_m beta[m,l] * homsol[m]  (kept in SBUF)
                    et = {}
                    for st in range(2):
                        homs = []
                        for mm in range(NZ):
                            h_t = homp.tile([128, NP], BF16, tag="hom",
                                            name=f"hom{st}{mm}")
                            nc.sync.dma_start(h_t[:], hom_in.ap()[mm, st])
                            homs.append(h_t)
                        for l in range(NZ):
                            e_t = epool.tile([128, NP], BF16,
                                             tag=f"e{l}{st}", name=f"e{l}{st}")
                            nc.scalar.mul(e_t[:], homs[0][:],
                                          brep[:, l:l + 1])
                            for mm in (1, 2, 3):
                                nc.vector.scalar_tensor_tensor(
                                    e_t[:], homs[mm][:],
                                    brep[:, 4 * mm + l:4 * mm + l + 1],
                                    e_t[:], MUL, ADD)
                            et[(l, st)] = e_t
                            # edge column x=2047 is correction-only
                            ec = homp.tile([128, 1], F32, tag="ec",
                                           name=f"ec{l}{st}")
                            nc.vector.tensor_copy(ec[:], e_t[:, 2047:2048])
                            nc.scalar.dma_start(
                                out_d.ap()[l, st, :, 2047:2048], ec[:])

                    # ---------------- stage 4 + postmix ----------------
                    with (
                        tc.tile_pool(name="u4", bufs=8) as u4,
                        tc.tile_pool(name="pstg", bufs=16) as pstg,
                        tc.tile_pool(name="tmppool", bufs=10) as tmp,
                    ):
                        ures = []
                        for m in range(NZ):
                            ures.append(
                                load_parity(u4, "u4", bout[2][m], f"u{m}"))
                        for st in range(2):
                            for n2 in range(2):
                                pv, pu = [], []
                                for m in range(NZ):
                                    pv.append(psum.tile(
                                        [128, 512], F32, tag="acc",
                                        name=f"s4v{m}"))
                                    pu.append(psum.tile(
                                        [128, 512], F32, tag="acc",
                                        name=f"s4u{m}"))
                                for m in range(NZ):
                                    te, to = ures[m]
                                    for k in range(HT):
                                        rq = PQ[:, 0, k,
                                                n2 * 512:(n2 + 1) * 512]
                                        rp = PQ[:, 1, k,
                                                n2 * 512:(n2 + 1) * 512]
                                        nc.tensor.matmul(
                                            pv[m][:],
                                            te[:, k, st * 128:(st + 1) * 128],
                                            rq, start=(k == 0),
                                            stop=(k == HT - 1))
                                        nc.tensor.matmul(
                                            pu[m][:],
                                            to[:, k, st * 128:(st + 1) * 128],
                                            rp, start=(k == 0),
                                            stop=(k == HT - 1))
                                pvb, pub = [], []
                                for m in range(NZ):
                                    a = pstg.tile([128, 512], BF16, tag="pst",
                                                  name=f"pvb{m}")
                                    nc.scalar.copy(a[:], pv[m][:])
                                    pvb.append(a)
                                    b = pstg.tile([128, 512], BF16, tag="pst",
                                                  name=f"pub{m}")
                                    nc.scalar.copy(b[:], pu[m][:])
                                    pub.append(b)
                                pss, psd = [], []
                                for m in range(NZ):
                                    s_ = pstg.tile([128, 512], BF16,
                                                   tag="pst", name=f"pss{m}")
                                    nc.vector.tensor_tensor(
                                        s_[:], pvb[m][:], pub[m][:], ADD)
                                    pss.append(s_)
                                    d_ = pstg.tile([128, 512], BF16,
                                                   tag="pst", name=f"psd{m}")
                                    nc.vector.tensor_tensor(
                                        d_[:], pvb[m][:], pub[m][:], SUB)
                                    psd.append(d_)
                                # sum side -> cols [n2*512, n2*512+512)
                                sl = slice(n2 * 512, (n2 + 1) * 512)
                                for l in range(NZ):
                                    t = tmp.tile([128, 512], BF16, tag="tmp",
                                                 name=f"ts{l}")
                                    nc.scalar.mul(t[:], pss[0][:],
                                                  cml[:, l:l + 1])
                                    for m in (1, 2, 3):
                                        nc.vector.scalar_tensor_tensor(
                                            t[:], pss[m][:],
                                            cml[:, 4 * m + l:4 * m + l + 1],
                                            t[:], MUL, ADD)
                                    t2 = tmp.tile([128, 512], F32, tag="tmpf",
                                                  name=f"ts2{l}")
                                    nc.vector.tensor_tensor(
                                        t2[:], t[:], et[(l, st)][:, sl], ADD)
                                    nc.sync.dma_start(
                                        out_d.ap()[l, st, :, sl], t2[:])
                                # diff side: n2=0 -> pos 2046..1535
                                #            n2=1 -> pos 1534..1024
                                cnt = 512 if n2 == 0 else 511
                                dsl = (slice(1535, 2047) if n2 == 0
                                       else slice(1024, 1535))
                                for l in range(NZ):
                                    t = tmp.tile([128, cnt], BF16, tag="tmp",
                                                 name=f"td{l}")
                                    nc.scalar.mul(t[:], psd[0][:, 0:cnt],
                                                  cml[:, l:l + 1])
                                    for m in (1, 2, 3):
                                        nc.vector.scalar_tensor_tensor(
                                            t[:], psd[m][:, 0:cnt],
                                            cml[:, 4 * m + l:4 * m + l + 1],
                                            t[:], MUL, ADD)
                                    tf = tmp.tile([128, cnt], F32,
                                                  tag="tmpf", name=f"tfd{l}")
                                    nc.vector.tensor_tensor(
                                        tf[:], t[:, ::-1],
                                        et[(l, st)][:, dsl], ADD)
                                    nc.sync.dma_start(
                                        out_d.ap()[l, st, :, dsl], tf[:])
    nc.compile()
    return nc


def _host_prep(q, Cl2m, Cm2l, H, homsol, homsol_mean):
    import ml_dtypes
    f32 = np.float32
    bf16 = ml_dtypes.bfloat16
    k = np.arange(1, NP, dtype=np.float64)
    S = np.sqrt(2.0 / NP) * np.sin(np.pi / NP * np.outer(k, k))
    Spad = np.zeros((NP, NP), f32)
    Spad[:N, :N] = S.astype(f32)
    w = np.zeros(NP, f32)
    w[:N] = S.sum(axis=0).astype(f32)

    # pq_in[0] = Q (even rows), pq_in[1] = P (odd rows)
    pq = np.stack([Spad[0::2, :NH], Spad[1::2, :NH]])
    pq = np.ascontiguousarray(pq.reshape(2, HT, 128, NH)).astype(bf16)

    qp = np.zeros((NZ, NP, NP), f32)
    qp[:, :N, :N] = q
    Hp = np.ones((NZ, NP, NP), f32)
    Hp[:, :N, :N] = H
    Hinv = (1.0 / Hp).astype(bf16)
    hom = np.ascontiguousarray(homsol[:, 1:NP + 1, 1:NP + 1]).astype(bf16)

    wrep = np.broadcast_to(w, (128, NP)).copy().astype(bf16)
    ones_r = np.ones((1, 128), f32)
    clm = np.broadcast_to(Cl2m.reshape(1, 16), (128, 16)).copy().astype(f32)
    cml = np.broadcast_to(Cm2l.T.reshape(1, 16), (128, 16)).copy().astype(f32)
    cmlT = np.ascontiguousarray(Cm2l.T).astype(f32)
    hm_i = homsol_mean.reshape(1, 4).astype(f32)

    in_maps = []
    for c in range(NC):
        ys = slice(c * SH, (c + 1) * SH)
        qs = qp[:, :, ys]  # [NZ, 2048, 256]
        qre = np.stack([qs[:, 0::2, :].reshape(NZ, HT, 128, SH),
                        qs[:, 1::2, :].reshape(NZ, HT, 128, SH)], axis=1)
        in_maps.append({
            "q_in": np.ascontiguousarray(qre),
            "pq_in": pq,
            "h_in": np.ascontiguousarray(
                Hinv[:, ys, :]).reshape(NZ, 2, 128, NP),
            "hom_in": np.ascontiguousarray(
                hom[:, ys, :]).reshape(NZ, 2, 128, NP),
            "wrep_in": wrep,
            "wcol_in": np.ascontiguousarray(w[ys].reshape(2, 128).T),
            "ones_in": ones_r,
            "clm_in": clm,
            "cml_in": cml,
            "cmlT_in": cmlT,
            "hm_in": hm_i,
        })
    return in_maps


def kernel(q, Cl2m, Cm2l, helmholtz_mat, homsol, homsol_mean,
           _want_results=False):
    from concourse.bass_utils import run_bass_kernel_spmd

    if "nc" not in _PROG:
        _PROG["nc"] = _build_program()
    nc = _PROG["nc"]

    in_maps = _host_prep(np.asarray(q, np.float32),
                         np.asarray(Cl2m, np.float32),
                         np.asarray(Cm2l, np.float32),
                         np.asarray(helmholtz_mat, np.float32),
                         np.asarray(homsol, np.float32),
                         np.asarray(homsol_mean, np.float32))
    res = run_bass_kernel_spmd(nc, in_maps, core_ids=list(range(NC)),
                               **_PROG.get("run_kwargs", {}))
    out = np.zeros((NZ, NP + 1, NP + 1), np.float32)
    for c in range(NC):
        core = res.results[c]["out_d"].reshape(NZ, SH, NP)
        out[:, 1 + c * SH:1 + (c + 1) * SH, 1:] = core
    beta = res.results[0]["beta_d"]  # beta[m, l] = alpha_m * Cm2l[l, m]
    hs = np.asarray(homsol, np.float32)
    out[:, 0, :] = np.einsum("ml,my->ly", beta, hs[:, 0, :])
    out[:, 1:, 0] = np.einsum("ml,mx->lx", beta, hs[:, 1:, 0])
    if _want_results:
        return out, res
    return out
